# revision 29
# baseline (speedup 1.0000x reference)
"""CSAEncoder Trainium2 kernel: 3-branch cross-attention + concat DoubleConv.

Sharding (8 cores): 2 batch groups x 4 tensor ranks.
Core c: batch b = c // 4, rank g = c % 4.
  - Attention: core computes heads [4g, 4g+4) of all 3 branches for batch b
    (a contiguous 128-channel slab of each branch's output).
  - conv1 computed as partial sums over the core's local 384 input channels
    for ALL 512 output channels; one chunked bf16 AllReduce(add) within the
    4-core batch group gives every rank the full conv1 pre-BN output.
  - conv2 computed locally: full 512-channel contraction, only the core's own
    128 output channels. No second collective.
Host assembles the full (2, 512, 32, 32) output from the 8 per-core slabs.

Schedule: the attention phase is paced by ACT's 96 softmax EXPs over
[128,1024] score tiles. Score matmuls for branch i are interleaved with
"filler" PE work (u projection, later branches' k/q projections, earlier
branches' conv1 partial blocks) so the tensor engine never idles waiting
for EXP results and stays at full p-state. Softmax normalization runs on
DVE: the pair's 4 denominator rows are copied to partitions {0,32,64,96},
one batched reciprocal amortizes the serial per-lane cost, stream_shuffle
broadcasts each quadrant row, one tensor_mul writes the padded xt slab
(channel bias xtb is folded into the value weights host-side via the
attn-rows-sum-to-one identity). A tiny "anchored barrier" AllReduce gated
on pair (2,0)'s output aligns the 4 cores before the real collective so
its peer-sync wait shrinks to residual skew.
"""

import os
import sys

import ml_dtypes
import numpy as np

for _p in ("/opt/trn_rl_repo",):
    if _p not in sys.path and os.path.isdir(_p):
        sys.path.insert(0, _p)

import concourse.bass as bass
import concourse.mybir as mybir
import concourse.tile as tile
from concourse import bacc
from concourse.bass_utils import run_bass_kernel_spmd

F32 = mybir.dt.float32
BF16 = mybir.dt.bfloat16
AF = mybir.ActivationFunctionType
BCAST0 = [0] * 32  # stream_shuffle mask: broadcast partition 0

B, C, H, W, HEADS = 2, 512, 32, 32, 16
D = C // HEADS            # 32
S = H * W                 # 1024
EPS = 1e-5
ISQD = 1.0 / np.sqrt(D)   # folded into the exp activation
NCORES = 8
GROUPS = [[0, 2, 4, 6], [1, 3, 5, 7]]
HP = W + 2                # padded row stride (34)


def build_nc():
    nc = bacc.Bacc(None, target_bir_lowering=False)

    # ---- per-core external inputs -------------------------------------
    x4_d = nc.declare_dram_parameter("x4", [4, 128, S], BF16, isOutput=False)
    oth_d = nc.declare_dram_parameter("oth", [2, 4, 128, S], BF16, isOutput=False)
    wqT_d = nc.declare_dram_parameter("wqT", [3, 4, 128, 128], BF16, isOutput=False)
    wkT_d = nc.declare_dram_parameter("wkT", [3, 4, 128, 128], BF16, isOutput=False)
    wvoT_d = nc.declare_dram_parameter("wvoT", [4, 128, 384], BF16, isOutput=False)
    dvec_d = nc.declare_dram_parameter("dvec", [128, 6], F32, isOutput=False)
    wobv_d = nc.declare_dram_parameter("wobv", [1, 384], F32, isOutput=False)
    c1wT_d = nc.declare_dram_parameter("c1wT", [3, 4, 128, 9, 128], BF16, isOutput=False)
    c2wT_d = nc.declare_dram_parameter("c2wT", [4, 128, 9, 128], BF16, isOutput=False)
    avec_d = nc.declare_dram_parameter("avec", [128, 10], F32, isOutput=False)
    out_d = nc.declare_dram_parameter("out", [128, S], F32, isOutput=True)

    with tile.TileContext(nc) as tc:
        import contextlib

        ctx = contextlib.ExitStack()
        with ctx:
            const = ctx.enter_context(tc.tile_pool(name="const", bufs=1))
            kq = ctx.enter_context(tc.tile_pool(name="kq", bufs=1))
            xtp = ctx.enter_context(tc.tile_pool(name="xtp", bufs=1))
            stg = ctx.enter_context(tc.tile_pool(name="stg", bufs=1))
            scps = ctx.enter_context(tc.tile_pool(name="scps", bufs=2, space="PSUM"))
            smps = ctx.enter_context(tc.tile_pool(name="smps", bufs=4, space="PSUM"))
            dram = ctx.enter_context(tc.tile_pool(name="dram", bufs=1, space="DRAM"))
            pt = ctx.enter_context(tc.tile_pool(name="pt", bufs=16))
            convw = ctx.enter_context(tc.tile_pool(name="convw", bufs=1))

            # ---- activations first (highest DMA priority) ----------------
            x_sb = const.tile([128, 4, S], BF16)
            oth_sb = const.tile([128, 2, 4, S], BF16)
            wq_sb = const.tile([128, 3, 4, 128], BF16)
            wk_sb = const.tile([128, 3, 4, 128], BF16)
            wvo_sb = const.tile([128, 4, 384], BF16)
            # priority order: branch-0 k needs x+wk0; q needs oth0+wq0; u
            # needs wvo; later branches' weights + x_prev follow
            for ks in range(4):
                nc.sync.dma_start(out=x_sb[:, ks, :], in_=x4_d[ks])
            for ks in range(4):
                nc.sync.dma_start(out=wk_sb[:, 0, ks, :], in_=wkT_d[0, ks])
            for ks in range(4):
                nc.sync.dma_start(out=oth_sb[:, 0, ks, :], in_=oth_d[0, ks])
            for ks in range(4):
                nc.sync.dma_start(out=wq_sb[:, 0, ks, :], in_=wqT_d[0, ks])
            for ks in range(4):
                nc.sync.dma_start(out=wvo_sb[:, ks, :], in_=wvoT_d[ks])
            for i in (1, 2):
                for ks in range(4):
                    nc.sync.dma_start(out=wk_sb[:, i, ks, :], in_=wkT_d[i, ks])
                    nc.sync.dma_start(out=wq_sb[:, i, ks, :], in_=wqT_d[i, ks])
            # x_prev: only needed for branch-2 q projection (filler work)
            for ks in range(4):
                nc.sync.dma_start(out=oth_sb[:, 1, ks, :], in_=oth_d[1, ks])

            # conv weights: needed from the middle of the attention phase on
            c1w_sb = [
                [convw.tile([128, 9, 128], BF16, name=f"c1w{i}_{m}") for m in range(4)]
                for i in range(3)
            ]
            c2w_sb = [convw.tile([128, 9, 128], BF16, name=f"c2w{k}") for k in range(4)]
            for i in range(3):
                for m in range(4):
                    nc.sync.dma_start(out=c1w_sb[i][m], in_=c1wT_d[i, m])
            for k in range(4):
                nc.sync.dma_start(out=c2w_sb[k], in_=c2wT_d[k])

            # Small consts: DMA to staging, then re-own on the consuming
            # engine (DVE / ACT) so consumers need no cross-engine const wait.
            dvec_st = const.tile([128, 6], F32)
            nc.gpsimd.dma_start(out=dvec_st, in_=dvec_d[:])
            wobv_st = const.tile([128, 384], F32)
            nc.gpsimd.dma_start(out=wobv_st, in_=wobv_d[:].partition_broadcast(128))
            avec_st = const.tile([128, 10], F32)
            nc.gpsimd.dma_start(out=avec_st, in_=avec_d[:])
            dvec = const.tile([128, 6], F32)
            nc.vector.tensor_copy(dvec, dvec_st)
            wobv_sb = const.tile([128, 384], F32)
            nc.vector.tensor_copy(wobv_sb, wobv_st)
            avec = const.tile([128, 10], F32)
            nc.scalar.activation(out=avec, in_=avec_st, func=AF.Copy)
            bqv_sb = dvec[:, 0:3]
            bkv_sb = dvec[:, 3:6]

            # xt (attention output) slabs + h1 slabs, zero-padded 34x34
            # (allocated here; memset emitted after P1 so the k/q bias adds
            # aren't queued behind them on DVE)
            xt_sl = [xtp.tile([128, HP, HP], BF16, name=f"xt{i}") for i in range(3)]
            h1_sl = [xtp.tile([128, HP, HP], BF16, name=f"h1{k}") for k in range(4)]
            # conv1 partial-sum accumulators (512 out ch as 4 m-tiles);
            # bf16 halves the DVE cost of the accumulate ops
            acc1 = [xtp.tile([128, S], BF16, name=f"acc1{m}") for m in range(4)]

            # softmax normalization scratch: the 4 denominator rows of a pair
            # are copied (GpSimd) to partitions {0,32,64,96} of dnT (engine
            # ops can only start on quadrant boundaries), one batched
            # reciprocal fills rcp, stream_shuffle broadcasts quadrant row 0
            dnT = stg.tile([128, S // 2], F32)
            nc.vector.memset(dnT, 1.0)
            rcp = stg.tile([128, S // 2], F32)
            rcb32 = [stg.tile([32, S // 2], F32, name=f"rcb32_{j}") for j in range(4)]

            # Semaphore warmers: absorb const-DMA + memset waits into each
            # engine's observed clock so later compute ops need <=1 wait.
            warm = const.tile([128, 1], F32)
            nc.vector.tensor_copy(warm, dvec[:, 0:1])
            warm2 = const.tile([128, 1], F32)
            nc.scalar.activation(out=warm2, in_=warm, func=AF.Copy)

            # k/q per branch (with biases added), uT tiles
            k_sb = kq.tile([128, 3, S], BF16)
            q_sb = kq.tile([128, 3, S], BF16)
            uT = [kq.tile([128, 3, 4, 33], BF16, name=f"uT{t}") for t in range(8)]
            qsrc = [oth_sb[:, 0], x_sb, oth_sb[:, 1]]

            # ---- phase P1: k/q for branch 0 (u moves to (0,0) fillers) ---
            for which, w_sb, src, bias in (
                ("k", wk_sb, x_sb, bkv_sb),
                ("q", wq_sb, qsrc[0], bqv_sb),
            ):
                ps = scps.tile([128, S], F32, name="kq_ps", tag="sc")
                for s in range(2):
                    for ks in range(4):
                        nc.tensor.matmul(
                            ps[:, 512 * s : 512 * (s + 1)],
                            lhsT=w_sb[:, 0, ks, :],
                            rhs=src[:, ks, 512 * s : 512 * (s + 1)],
                            start=(ks == 0),
                            stop=(ks == 3),
                        )
                dst = k_sb if which == "k" else q_sb
                nc.vector.tensor_scalar_add(dst[:, 0, :], ps, bias[:, 0:1])

            def u_unit(tu):
                for t in (2 * tu, 2 * tu + 1):
                    u_ps = smps.tile([128, 384], F32, name="u_ps", tag="sm")
                    for ks in range(4):
                        nc.tensor.matmul(
                            u_ps,
                            lhsT=x_sb[:, ks, 128 * t : 128 * (t + 1)],
                            rhs=wvo_sb[:, ks, :],
                            start=(ks == 0),
                            stop=(ks == 3),
                        )
                    nc.vector.memset(uT[t][:, :, :, 32:33], 1.0)
                    nc.vector.tensor_add(
                        uT[t][:, :, :, 0:32],
                        u_ps.rearrange("p (i h d) -> p i h d", i=3, h=4),
                        wobv_sb.rearrange("p (i h d) -> p i h d", i=3, h=4),
                    )

            # ---- filler units (PE work with no EXP dependency) -----------
            def proj_half(i, which, s):
                """k or q projection for branch i, spatial half s (4 matmuls
                into a 1-bank PSUM tile + one DVE bias-add)."""
                w_sb, src, bias, dst = (
                    (wk_sb, x_sb, bkv_sb, k_sb)
                    if which == "k"
                    else (wq_sb, qsrc[i], bqv_sb, q_sb)
                )
                ps = smps.tile([128, 512], F32, name="pj", tag="sm")
                for ks in range(4):
                    nc.tensor.matmul(
                        ps,
                        lhsT=w_sb[:, i, ks, :],
                        rhs=src[:, ks, 512 * s : 512 * (s + 1)],
                        start=(ks == 0),
                        stop=(ks == 3),
                    )
                nc.vector.tensor_scalar_add(
                    dst[:, i, 512 * s : 512 * (s + 1)], ps, bias[:, i : i + 1]
                )

            def conv1_block(i, m, n, final_dst=None):
                """Partial conv1 for xt slab i, out m-tile, spatial half n,
                accumulated into acc1[m] (or bf16 final_dst when i==2)."""
                ps = smps.tile([128, 512], F32, name="cv", tag="sm")
                for dy in range(3):
                    for dx in range(3):
                        nc.tensor.matmul(
                            ps,
                            lhsT=c1w_sb[i][m][:, dy * 3 + dx, :],
                            rhs=xt_sl[i][:, 16 * n + dy : 16 * n + dy + 16, dx : dx + 32],
                            start=(dy == 0 and dx == 0),
                            stop=(dy == 2 and dx == 2),
                        )
                if i == 0:
                    nc.vector.tensor_copy(acc1[m][:, 512 * n : 512 * (n + 1)], ps)
                elif i == 1:
                    dst = acc1[m][:, 512 * n : 512 * (n + 1)]
                    nc.vector.tensor_add(dst, ps, dst)
                else:
                    nc.vector.tensor_add(
                        final_dst[:, 512 * n : 512 * (n + 1)],
                        ps,
                        acc1[m][:, 512 * n : 512 * (n + 1)],
                    )

            FILL = {
                (0, 0): [lambda tu=tu: u_unit(tu) for tu in range(4)]
                + [lambda s=s, w=w: proj_half(1, w, s) for w in ("k", "q") for s in range(2)],
                (0, 1): [lambda s=s, w=w: proj_half(2, w, s) for w in ("k", "q") for s in range(2)],
                (1, 0): [lambda m=m, n=n: conv1_block(0, m, n) for m in range(2) for n in range(2)],
                (1, 1): [lambda m=m, n=n: conv1_block(0, m, n) for m in (2, 3) for n in range(2)],
                (2, 0): [lambda m=m, n=n: conv1_block(1, m, n) for m in range(2) for n in range(2)],
                (2, 1): [lambda m=m, n=n: conv1_block(1, m, n) for m in (2, 3) for n in range(2)],
            }

            drum_in = [dram.tile([1, 16], BF16, name=f"drum_in{a}") for a in range(2)]
            drum_out = [dram.tile([1, 16], BF16, name=f"drum_out{a}") for a in range(2)]
            for t_ in xt_sl + h1_sl:
                nc.vector.memset(t_, 0.0)

            # ---- phase A: attention, EXP-paced with interleaved filler ---
            for i in range(3):
                for pr in range(2):
                    heads = (2 * pr, 2 * pr + 1)
                    fillers = FILL[(i, pr)]
                    fi = 0
                    pts = {}
                    for t in range(8):
                        for h in heads:
                            sc = scps.tile([128, S], F32, name="sc", tag="sc")
                            p0 = 32 * h
                            for s in range(2):
                                nc.tensor.matmul(
                                    sc[:, 512 * s : 512 * (s + 1)],
                                    lhsT=k_sb[p0 : p0 + 32, i, 128 * t : 128 * (t + 1)],
                                    rhs=q_sb[p0 : p0 + 32, i, 512 * s : 512 * (s + 1)],
                                    start=True,
                                    stop=True,
                                    tile_position=(p0, 0),
                                )
                            ptt = pt.tile([128, S], BF16, name="ptt")
                            nc.scalar.activation(
                                out=ptt, in_=sc, func=AF.Exp, scale=float(ISQD)
                            )
                            pts[(h, t)] = ptt
                        last_pair = (i, pr) == (2, 1)
                        if t >= 1 and fi < len(fillers) - (1 if last_pair else 0):
                            fillers[fi]()
                            fi += 1
                    ys = {}
                    for hi, h in enumerate(heads):
                        for s in range(2):
                            y = smps.tile([33, 512], F32, name="y", tag="sm")
                            for t in range(8):
                                nc.tensor.matmul(
                                    y,
                                    lhsT=uT[t][:, i, h, :],
                                    rhs=pts[(h, t)][:, 512 * s : 512 * (s + 1)],
                                    start=(t == 0),
                                    stop=(t == 7),
                                )
                            j = 2 * hi + s
                            nc.vector.tensor_copy(dnT[32 * j : 32 * j + 1, :], y[32:33, :])
                            # evacuate y from PSUM immediately so the bank
                            # recycles without waiting on the reciprocal chain
                            ysb = stg.tile([32, 512], F32, name="ysb", bufs=4)
                            nc.vector.tensor_copy(ysb, y[0:32, :])
                            ys[(hi, s)] = ysb
                    nc.vector.reciprocal(rcp, dnT)
                    for hi, h in enumerate(heads):
                        p0 = 32 * h
                        for s in range(2):
                            j = 2 * hi + s
                            nc.vector.stream_shuffle(
                                rcb32[j], rcp[32 * j : 32 * j + 32, :], mask=BCAST0
                            )
                            # all-SBUF multiply runs on the idle Pool engine,
                            # overlapping the next stream_shuffle on DVE.
                            # For the last pair's s=1 halves, write spatial
                            # row 17 first: conv1(2) n=0 blocks need only
                            # rows 0..17, so they can start ~3us earlier.
                            split = i == 2 and s == 1
                            if split:
                                nc.gpsimd.tensor_mul(
                                    xt_sl[i][p0 : p0 + 32, 17:18, 1:33],
                                    ys[(hi, s)][0:32, 0:32].rearrange("p (a b) -> p a b", b=32),
                                    rcb32[j][:, 0:32].rearrange("p (a b) -> p a b", b=32),
                                )
                                nc.gpsimd.tensor_mul(
                                    xt_sl[i][p0 : p0 + 32, 18:33, 1:33],
                                    ys[(hi, s)][0:32, 32:512].rearrange("p (a b) -> p a b", b=32),
                                    rcb32[j][:, 32:512].rearrange("p (a b) -> p a b", b=32),
                                )
                            else:
                                nc.gpsimd.tensor_mul(
                                    xt_sl[i][p0 : p0 + 32, 1 + 16 * s : 17 + 16 * s, 1:33],
                                    ys[(hi, s)][0:32, :].rearrange("p (a b) -> p a b", b=32),
                                    rcb32[j].rearrange("p (a b) -> p a b", b=32),
                                )
                    while fi < len(fillers):
                        fillers[fi]()
                        fi += 1
                    if (i, pr) == (2, 1):
                        # anchored barrier: tiny AllReduce gated on this
                        # pair's first xt write aligns the 4 cores ~10us
                        # before the real collective, so the big AllReduce's
                        # peer-sync wait shrinks to residual skew
                        nc.sync.dma_start(out=drum_in[0][:], in_=xt_sl[2][64:65, 1, 1:17])
                        nc.gpsimd.collective_compute(
                            "AllReduce",
                            mybir.AluOpType.add,
                            replica_groups=GROUPS,
                            ins=[drum_in[0][:]],
                            outs=[drum_out[0][:]],
                        )

            # ---- phase C: conv1 tail, AllReduce, BN1, conv2, out ---------
            AR_DT = BF16
            p1sb = [stg.tile([128, S], AR_DT, name=f"p1sb{m}") for m in range(4)]
            partial1 = [dram.tile([256, S], AR_DT, name=f"partial1{a}") for a in range(2)]
            art = [dram.tile([256, S], AR_DT, name=f"art{a}") for a in range(2)]

            def ar_kick(a):
                for m in (2 * a, 2 * a + 1):
                    nc.sync.dma_start(
                        out=partial1[a][(m % 2) * 128 : (m % 2) * 128 + 128, :],
                        in_=p1sb[m],
                    )
                nc.gpsimd.collective_compute(
                    "AllReduce",
                    mybir.AluOpType.add,
                    replica_groups=GROUPS,
                    ins=[partial1[a][:]],
                    outs=[art[a][:]],
                )

            for m in range(2):
                conv1_block(2, m, 0, final_dst=p1sb[m])
            for m in range(2):
                conv1_block(2, m, 1, final_dst=p1sb[m])
            ar_kick(0)
            for m in range(2, 4):
                conv1_block(2, m, 0, final_dst=p1sb[m])
            for m in range(2, 4):
                conv1_block(2, m, 1, final_dst=p1sb[m])
            ar_kick(1)

            arraw = stg.tile([128, 4, S], AR_DT, name="arraw")
            oout = stg.tile([128, S], F32, name="oout")
            ps2 = [smps.tile([128, 512], F32, name=f"cv2_{n}", tag="sm") for n in range(2)]

            def h1_make(k):
                nc.sync.dma_start(
                    out=arraw[:, k, :],
                    in_=art[k // 2][(k % 2) * 128 : (k % 2) * 128 + 128, :],
                )
                nc.scalar.activation(
                    out=h1_sl[k][:, 1:33, 1:33],
                    in_=arraw[:, k, :].rearrange("p (a b) -> p a b", b=32),
                    func=AF.Relu,
                    bias=avec[:, 4 + k : 5 + k],
                    scale=avec[:, k : k + 1],
                )

            def conv2_k(k):
                for n in range(2):
                    for dy in range(3):
                        for dx in range(3):
                            nc.tensor.matmul(
                                ps2[n],
                                lhsT=c2w_sb[k][:, dy * 3 + dx, :],
                                rhs=h1_sl[k][
                                    :, 16 * n + dy : 16 * n + dy + 16, dx : dx + 32
                                ],
                                start=(k == 0 and dy == 0 and dx == 0),
                                stop=(k == 3 and dy == 2 and dx == 2),
                            )

            for k in range(4):
                h1_make(k)
                conv2_k(k)
            for n in range(2):
                nc.scalar.activation(
                    out=oout[:, 512 * n : 512 * (n + 1)],
                    in_=ps2[n],
                    func=AF.Relu,
                    bias=avec[:, 9:10],
                    scale=avec[:, 8:9],
                )
                nc.sync.dma_start(
                    out=out_d[:, 512 * n : 512 * (n + 1)],
                    in_=oout[:, 512 * n : 512 * (n + 1)],
                )

    nc.finalize()
    return nc


def _f(x):
    return np.ascontiguousarray(x, dtype=np.float32)


def _bf(x):
    return np.ascontiguousarray(np.asarray(x, dtype=np.float32).astype(ml_dtypes.bfloat16))


def prepare_core_inputs(inp):
    """Build the 8 per-core input dicts from the full-problem inputs."""
    inp = {k: np.asarray(v, dtype=np.float64) for k, v in inp.items()}
    x = inp["x"].reshape(B, C, S)
    xp = inp["x_prev"].reshape(B, C, S)
    xn = inp["x_next"].reshape(B, C, S)

    bn1s_full = inp["bn1g"] / np.sqrt(inp["bn1v"] + EPS)
    bn1b_full = inp["bn1b"] - inp["bn1m"] * bn1s_full
    bn2s_full = inp["bn2g"] / np.sqrt(inp["bn2v"] + EPS)
    bn2b_full = inp["bn2b"] - inp["bn2m"] * bn2s_full

    per_g = []
    for g in range(4):
        sl = slice(128 * g, 128 * (g + 1))
        wqT = np.stack(
            [
                np.stack([inp["Wq"][i][sl, 128 * k : 128 * (k + 1)].T for k in range(4)])
                for i in range(3)
            ]
        )
        wkT = np.stack(
            [
                np.stack([inp["Wk"][i][sl, 128 * k : 128 * (k + 1)].T for k in range(4)])
                for i in range(3)
            ]
        )
        bqv = np.stack([inp["bq"][i][sl] for i in range(3)], axis=1)
        bkv = np.stack([inp["bk"][i][sl] for i in range(3)], axis=1)

        att_s = np.stack(
            [inp["bng"][i][sl] / np.sqrt(inp["bnv"][i][sl] + EPS) for i in range(3)]
        )  # (3,128)
        # xtb: post-BN channel bias, folded into the value weights (the
        # attention rows sum to 1 after normalization, so adding xtb to every
        # key position of u produces  y/den + xtb  for free)
        xtb = np.stack(
            [
                inp["bnb"][i][sl] + (inp["bo"][i][sl] - inp["bnm"][i][sl]) * att_s[i]
                for i in range(3)
            ]
        )  # (3,128)

        wvo_rows = []
        wobv_row = []
        for i in range(3):
            for hl in range(4):
                hg = 4 * g + hl
                wv_h = inp["Wv"][i][32 * hg : 32 * (hg + 1), :]  # (32, 512)
                bv_h = inp["bv"][i][32 * hg : 32 * (hg + 1)]
                wo_h = inp["Wo"][i, hg]  # (32, 32)
                sc = att_s[i][32 * hl : 32 * (hl + 1)]  # (32,)
                wvo_rows.append(sc[:, None] * (wo_h @ wv_h))
                wobv_row.append(sc * (wo_h @ bv_h) + xtb[i][32 * hl : 32 * (hl + 1)])
        wvo_all = np.concatenate(wvo_rows, axis=0)  # (384, 512)
        wobv = np.concatenate(wobv_row)[None, :]  # (1, 384)
        wvoT = np.stack([wvo_all[:, 128 * k : 128 * (k + 1)].T for k in range(4)])

        c1wT = np.stack(
            [
                np.stack(
                    [
                        inp["c1w"][
                            128 * m : 128 * (m + 1),
                            512 * i + 128 * g : 512 * i + 128 * (g + 1),
                        ]
                        .transpose(1, 2, 3, 0)
                        .reshape(128, 9, 128)
                        for m in range(4)
                    ]
                )
                for i in range(3)
            ]
        )
        c2wT = np.stack(
            [
                inp["c2w"][sl, 128 * k : 128 * (k + 1)]
                .transpose(1, 2, 3, 0)
                .reshape(128, 9, 128)
                for k in range(4)
            ]
        )
        avec = np.concatenate(
            [
                bn1s_full.reshape(4, 128).T,
                bn1b_full.reshape(4, 128).T,
                bn2s_full[sl][:, None],
                bn2b_full[sl][:, None],
            ],
            axis=1,
        )  # (128, 10)

        per_g.append(
            dict(
                wqT=_bf(wqT), wkT=_bf(wkT), wvoT=_bf(wvoT),
                wobv=_f(wobv), c1wT=_bf(c1wT), c2wT=_bf(c2wT),
                dvec=_f(np.concatenate([bqv, bkv], axis=1)),
                avec=_f(avec),
            )
        )

    in_maps = []
    for c in range(NCORES):
        b, g = c % 2, c // 2
        d = dict(per_g[g])
        d["x4"] = _bf(x[b].reshape(4, 128, S))
        d["oth"] = _bf(np.stack([xn[b].reshape(4, 128, S), xp[b].reshape(4, 128, S)]))
        in_maps.append(d)
    return in_maps


_NC_CACHE = {}


def get_nc():
    if "nc" not in _NC_CACHE:
        _NC_CACHE["nc"] = build_nc()
    return _NC_CACHE["nc"]


def assemble(results):
    out = np.zeros((B, C, H, W), dtype=np.float32)
    for c in range(NCORES):
        b, g = c % 2, c // 2
        out[b, 128 * g : 128 * (g + 1)] = results[c]["out"].reshape(128, H, W)
    return out


def kernel(**inputs):
    nc = get_nc()
    in_maps = prepare_core_inputs(inputs)
    res = run_bass_kernel_spmd(nc, in_maps, list(range(NCORES)))
    return assemble(res.results)


# revision 30
# speedup vs baseline: 1.0197x; 1.0197x over previous
"""CSAEncoder Trainium2 kernel: 3-branch cross-attention + concat DoubleConv.

Sharding (8 cores): 2 batch groups x 4 tensor ranks.
Core c: batch b = c // 4, rank g = c % 4.
  - Attention: core computes heads [4g, 4g+4) of all 3 branches for batch b
    (a contiguous 128-channel slab of each branch's output).
  - conv1 computed as partial sums over the core's local 384 input channels
    for ALL 512 output channels; one chunked bf16 AllReduce(add) within the
    4-core batch group gives every rank the full conv1 pre-BN output.
  - conv2 computed locally: full 512-channel contraction, only the core's own
    128 output channels. No second collective.
Host assembles the full (2, 512, 32, 32) output from the 8 per-core slabs.

Schedule: the attention phase is paced by ACT's 96 softmax EXPs over
[128,1024] score tiles. Score matmuls for branch i are interleaved with
"filler" PE work (u projection, later branches' k/q projections, earlier
branches' conv1 partial blocks) so the tensor engine never idles waiting
for EXP results and stays at full p-state. Softmax normalization runs on
DVE: the pair's 4 denominator rows are copied to partitions {0,32,64,96},
one batched reciprocal amortizes the serial per-lane cost, stream_shuffle
broadcasts each quadrant row, one tensor_mul writes the padded xt slab
(channel bias xtb is folded into the value weights host-side via the
attn-rows-sum-to-one identity). A tiny "anchored barrier" AllReduce gated
on pair (2,0)'s output aligns the 4 cores before the real collective so
its peer-sync wait shrinks to residual skew.
"""

import os
import sys

import ml_dtypes
import numpy as np

for _p in ("/opt/trn_rl_repo",):
    if _p not in sys.path and os.path.isdir(_p):
        sys.path.insert(0, _p)

import concourse.bass as bass
import concourse.mybir as mybir
import concourse.tile as tile
from concourse import bacc
from concourse.bass_utils import run_bass_kernel_spmd

F32 = mybir.dt.float32
BF16 = mybir.dt.bfloat16
AF = mybir.ActivationFunctionType
BCAST0 = [0] * 32  # stream_shuffle mask: broadcast partition 0

B, C, H, W, HEADS = 2, 512, 32, 32, 16
D = C // HEADS            # 32
S = H * W                 # 1024
EPS = 1e-5
ISQD = 1.0 / np.sqrt(D)   # folded into the exp activation
NCORES = 8
GROUPS = [[0, 2, 4, 6], [1, 3, 5, 7]]
HP = W + 2                # padded row stride (34)


def build_nc():
    nc = bacc.Bacc(None, target_bir_lowering=False)

    # ---- per-core external inputs -------------------------------------
    x4_d = nc.declare_dram_parameter("x4", [4, 128, S], BF16, isOutput=False)
    oth_d = nc.declare_dram_parameter("oth", [2, 4, 128, S], BF16, isOutput=False)
    wqT_d = nc.declare_dram_parameter("wqT", [3, 4, 128, 128], BF16, isOutput=False)
    wkT_d = nc.declare_dram_parameter("wkT", [3, 4, 128, 128], BF16, isOutput=False)
    wvoT_d = nc.declare_dram_parameter("wvoT", [4, 128, 384], BF16, isOutput=False)
    dvec_d = nc.declare_dram_parameter("dvec", [128, 6], F32, isOutput=False)
    wobv_d = nc.declare_dram_parameter("wobv", [1, 384], F32, isOutput=False)
    c1wT_d = nc.declare_dram_parameter("c1wT", [3, 4, 128, 9, 128], BF16, isOutput=False)
    c2wT_d = nc.declare_dram_parameter("c2wT", [4, 128, 9, 128], BF16, isOutput=False)
    avec_d = nc.declare_dram_parameter("avec", [128, 10], F32, isOutput=False)
    out_d = nc.declare_dram_parameter("out", [128, S], F32, isOutput=True)

    with tile.TileContext(nc) as tc:
        import contextlib

        ctx = contextlib.ExitStack()
        with ctx:
            const = ctx.enter_context(tc.tile_pool(name="const", bufs=1))
            kq = ctx.enter_context(tc.tile_pool(name="kq", bufs=1))
            xtp = ctx.enter_context(tc.tile_pool(name="xtp", bufs=1))
            stg = ctx.enter_context(tc.tile_pool(name="stg", bufs=1))
            scps = ctx.enter_context(tc.tile_pool(name="scps", bufs=2, space="PSUM"))
            smps = ctx.enter_context(tc.tile_pool(name="smps", bufs=4, space="PSUM"))
            dram = ctx.enter_context(tc.tile_pool(name="dram", bufs=1, space="DRAM"))
            pt = ctx.enter_context(tc.tile_pool(name="pt", bufs=16))
            convw = ctx.enter_context(tc.tile_pool(name="convw", bufs=1))

            # ---- activations first (highest DMA priority) ----------------
            x_sb = const.tile([128, 4, S], BF16)
            oth_sb = const.tile([128, 2, 4, S], BF16)
            wq_sb = const.tile([128, 3, 4, 128], BF16)
            wk_sb = const.tile([128, 3, 4, 128], BF16)
            wvo_sb = const.tile([128, 4, 384], BF16)
            # priority order: branch-0 k needs x+wk0; q needs oth0+wq0; u
            # needs wvo; later branches' weights + x_prev follow
            for ks in range(4):
                nc.sync.dma_start(out=x_sb[:, ks, :], in_=x4_d[ks])
            for ks in range(4):
                nc.sync.dma_start(out=wk_sb[:, 0, ks, :], in_=wkT_d[0, ks])
            for ks in range(4):
                nc.sync.dma_start(out=oth_sb[:, 0, ks, :], in_=oth_d[0, ks])
            for ks in range(4):
                nc.sync.dma_start(out=wq_sb[:, 0, ks, :], in_=wqT_d[0, ks])
            for ks in range(4):
                nc.sync.dma_start(out=wvo_sb[:, ks, :], in_=wvoT_d[ks])
            for i in (1, 2):
                for ks in range(4):
                    nc.sync.dma_start(out=wk_sb[:, i, ks, :], in_=wkT_d[i, ks])
                    nc.sync.dma_start(out=wq_sb[:, i, ks, :], in_=wqT_d[i, ks])
            # x_prev: only needed for branch-2 q projection (filler work)
            for ks in range(4):
                nc.sync.dma_start(out=oth_sb[:, 1, ks, :], in_=oth_d[1, ks])

            # conv weights: needed from the middle of the attention phase on
            c1w_sb = [
                [convw.tile([128, 9, 128], BF16, name=f"c1w{i}_{m}") for m in range(4)]
                for i in range(3)
            ]
            c2w_sb = [convw.tile([128, 9, 128], BF16, name=f"c2w{k}") for k in range(4)]
            for i in range(3):
                for m in range(4):
                    nc.sync.dma_start(out=c1w_sb[i][m], in_=c1wT_d[i, m])
            for k in range(4):
                nc.sync.dma_start(out=c2w_sb[k], in_=c2wT_d[k])

            # Small consts: DMA to staging, then re-own on the consuming
            # engine (DVE / ACT) so consumers need no cross-engine const wait.
            dvec_st = const.tile([128, 6], F32)
            nc.gpsimd.dma_start(out=dvec_st, in_=dvec_d[:])
            wobv_st = const.tile([128, 384], F32)
            nc.gpsimd.dma_start(out=wobv_st, in_=wobv_d[:].partition_broadcast(128))
            avec_st = const.tile([128, 10], F32)
            nc.gpsimd.dma_start(out=avec_st, in_=avec_d[:])
            dvec = const.tile([128, 6], F32)
            nc.vector.tensor_copy(dvec, dvec_st)
            wobv_sb = const.tile([128, 384], F32)
            nc.vector.tensor_copy(wobv_sb, wobv_st)
            avec = const.tile([128, 10], F32)
            nc.scalar.activation(out=avec, in_=avec_st, func=AF.Copy)
            bqv_sb = dvec[:, 0:3]
            bkv_sb = dvec[:, 3:6]

            # xt (attention output) slabs + h1 slabs, zero-padded 34x34
            # (allocated here; memset emitted after P1 so the k/q bias adds
            # aren't queued behind them on DVE)
            xt_sl = [xtp.tile([128, HP, HP], BF16, name=f"xt{i}") for i in range(3)]
            h1_sl = [xtp.tile([128, HP, HP], BF16, name=f"h1{k}") for k in range(4)]
            # conv1 partial-sum accumulators (512 out ch as 4 m-tiles);
            # bf16 halves the DVE cost of the accumulate ops
            acc1 = [xtp.tile([128, S], BF16, name=f"acc1{m}") for m in range(4)]

            # softmax normalization scratch: the 4 denominator rows of a pair
            # are copied (GpSimd) to partitions {0,32,64,96} of dnT (engine
            # ops can only start on quadrant boundaries), one batched
            # reciprocal fills rcp, stream_shuffle broadcasts quadrant row 0
            dnT = stg.tile([128, S // 2], F32)
            nc.vector.memset(dnT, 1.0)
            rcp = stg.tile([128, S // 2], F32)
            rcb32 = [stg.tile([32, S // 2], F32, name=f"rcb32_{j}") for j in range(4)]

            # Semaphore warmers: absorb const-DMA + memset waits into each
            # engine's observed clock so later compute ops need <=1 wait.
            warm = const.tile([128, 1], F32)
            nc.vector.tensor_copy(warm, dvec[:, 0:1])
            warm2 = const.tile([128, 1], F32)
            nc.scalar.activation(out=warm2, in_=warm, func=AF.Copy)

            # k/q per branch (with biases added), uT tiles
            k_sb = kq.tile([128, 3, S], BF16)
            q_sb = kq.tile([128, 3, S], BF16)
            uT = [kq.tile([128, 3, 4, 33], BF16, name=f"uT{t}") for t in range(8)]
            qsrc = [oth_sb[:, 0], x_sb, oth_sb[:, 1]]

            # ---- phase P1: k/q for branch 0 (u moves to (0,0) fillers) ---
            for which, w_sb, src, bias in (
                ("k", wk_sb, x_sb, bkv_sb),
                ("q", wq_sb, qsrc[0], bqv_sb),
            ):
                ps = scps.tile([128, S], F32, name="kq_ps", tag="sc")
                for s in range(2):
                    for ks in range(4):
                        nc.tensor.matmul(
                            ps[:, 512 * s : 512 * (s + 1)],
                            lhsT=w_sb[:, 0, ks, :],
                            rhs=src[:, ks, 512 * s : 512 * (s + 1)],
                            start=(ks == 0),
                            stop=(ks == 3),
                        )
                dst = k_sb if which == "k" else q_sb
                nc.vector.tensor_scalar_add(dst[:, 0, :], ps, bias[:, 0:1])

            def u_unit(tu):
                for t in (2 * tu, 2 * tu + 1):
                    u_ps = smps.tile([128, 384], F32, name="u_ps", tag="sm")
                    for ks in range(4):
                        nc.tensor.matmul(
                            u_ps,
                            lhsT=x_sb[:, ks, 128 * t : 128 * (t + 1)],
                            rhs=wvo_sb[:, ks, :],
                            start=(ks == 0),
                            stop=(ks == 3),
                        )
                    nc.vector.memset(uT[t][:, :, :, 32:33], 1.0)
                    nc.vector.tensor_add(
                        uT[t][:, :, :, 0:32],
                        u_ps.rearrange("p (i h d) -> p i h d", i=3, h=4),
                        wobv_sb.rearrange("p (i h d) -> p i h d", i=3, h=4),
                    )

            # ---- filler units (PE work with no EXP dependency) -----------
            def proj_half(i, which, s):
                """k or q projection for branch i, spatial half s (4 matmuls
                into a 1-bank PSUM tile + one DVE bias-add)."""
                w_sb, src, bias, dst = (
                    (wk_sb, x_sb, bkv_sb, k_sb)
                    if which == "k"
                    else (wq_sb, qsrc[i], bqv_sb, q_sb)
                )
                ps = smps.tile([128, 512], F32, name="pj", tag="sm")
                for ks in range(4):
                    nc.tensor.matmul(
                        ps,
                        lhsT=w_sb[:, i, ks, :],
                        rhs=src[:, ks, 512 * s : 512 * (s + 1)],
                        start=(ks == 0),
                        stop=(ks == 3),
                    )
                nc.vector.tensor_scalar_add(
                    dst[:, i, 512 * s : 512 * (s + 1)], ps, bias[:, i : i + 1]
                )

            def conv1_block(i, m, n, final_dst=None):
                """Partial conv1 for xt slab i, out m-tile, spatial half n,
                accumulated into acc1[m] (or bf16 final_dst when i==2)."""
                ps = smps.tile([128, 512], F32, name="cv", tag="sm")
                for dy in range(3):
                    for dx in range(3):
                        nc.tensor.matmul(
                            ps,
                            lhsT=c1w_sb[i][m][:, dy * 3 + dx, :],
                            rhs=xt_sl[i][:, 16 * n + dy : 16 * n + dy + 16, dx : dx + 32],
                            start=(dy == 0 and dx == 0),
                            stop=(dy == 2 and dx == 2),
                        )
                if i == 0:
                    nc.vector.tensor_copy(acc1[m][:, 512 * n : 512 * (n + 1)], ps)
                elif i == 1:
                    dst = acc1[m][:, 512 * n : 512 * (n + 1)]
                    nc.vector.tensor_add(dst, ps, dst)
                else:
                    nc.vector.tensor_add(
                        final_dst[:, 512 * n : 512 * (n + 1)],
                        ps,
                        acc1[m][:, 512 * n : 512 * (n + 1)],
                    )

            FILL = {
                (0, 0): [lambda tu=tu: u_unit(tu) for tu in range(4)]
                + [lambda s=s, w=w: proj_half(1, w, s) for w in ("k", "q") for s in range(2)],
                (0, 1): [lambda s=s, w=w: proj_half(2, w, s) for w in ("k", "q") for s in range(2)],
                (1, 0): [lambda m=m, n=n: conv1_block(0, m, n) for m in range(2) for n in range(2)],
                (1, 1): [lambda m=m, n=n: conv1_block(0, m, n) for m in (2, 3) for n in range(2)],
                (2, 0): [lambda m=m, n=n: conv1_block(1, m, n) for m in range(2) for n in range(2)],
                (2, 1): [lambda m=m, n=n: conv1_block(1, m, n) for m in (2, 3) for n in range(2)],
            }

            drum_in = [dram.tile([1, 16], BF16, name=f"drum_in{a}") for a in range(2)]
            drum_out = [dram.tile([1, 16], BF16, name=f"drum_out{a}") for a in range(2)]
            for t_ in xt_sl + h1_sl:
                nc.vector.memset(t_, 0.0)

            # ---- phase A: attention, EXP-paced with interleaved filler ---
            for i in range(3):
                for pr in range(2):
                    heads = (2 * pr, 2 * pr + 1)
                    fillers = FILL[(i, pr)]
                    fi = 0
                    pts = {}
                    for t in range(8):
                        for h in heads:
                            sc = scps.tile([128, S], F32, name="sc", tag="sc")
                            p0 = 32 * h
                            for s in range(2):
                                nc.tensor.matmul(
                                    sc[:, 512 * s : 512 * (s + 1)],
                                    lhsT=k_sb[p0 : p0 + 32, i, 128 * t : 128 * (t + 1)],
                                    rhs=q_sb[p0 : p0 + 32, i, 512 * s : 512 * (s + 1)],
                                    start=True,
                                    stop=True,
                                    tile_position=(p0, 0),
                                )
                            ptt = pt.tile([128, S], BF16, name="ptt")
                            nc.scalar.activation(
                                out=ptt, in_=sc, func=AF.Exp, scale=float(ISQD)
                            )
                            pts[(h, t)] = ptt
                        last_pair = (i, pr) == (2, 1)
                        if t >= 1 and fi < len(fillers) - (1 if last_pair else 0):
                            fillers[fi]()
                            fi += 1
                    ys = {}
                    for hi, h in enumerate(heads):
                        for s in range(2):
                            y = smps.tile([33, 512], F32, name="y", tag="sm")
                            for t in range(8):
                                nc.tensor.matmul(
                                    y,
                                    lhsT=uT[t][:, i, h, :],
                                    rhs=pts[(h, t)][:, 512 * s : 512 * (s + 1)],
                                    start=(t == 0),
                                    stop=(t == 7),
                                )
                            j = 2 * hi + s
                            nc.vector.tensor_copy(dnT[32 * j : 32 * j + 1, :], y[32:33, :])
                            # evacuate y from PSUM immediately so the bank
                            # recycles without waiting on the reciprocal chain
                            ysb = stg.tile([32, 512], F32, name="ysb", bufs=4)
                            nc.vector.tensor_copy(ysb, y[0:32, :])
                            ys[(hi, s)] = ysb
                    nc.vector.reciprocal(rcp, dnT)
                    for hi, h in enumerate(heads):
                        p0 = 32 * h
                        for s in range(2):
                            j = 2 * hi + s
                            nc.vector.stream_shuffle(
                                rcb32[j], rcp[32 * j : 32 * j + 32, :], mask=BCAST0
                            )
                            # all-SBUF multiply runs on the idle Pool engine,
                            # overlapping the next stream_shuffle on DVE
                            nc.gpsimd.tensor_mul(
                                xt_sl[i][p0 : p0 + 32, 1 + 16 * s : 17 + 16 * s, 1:33],
                                ys[(hi, s)][0:32, :].rearrange("p (a b) -> p a b", b=32),
                                rcb32[j].rearrange("p (a b) -> p a b", b=32),
                            )
                    while fi < len(fillers):
                        fillers[fi]()
                        fi += 1
                    if (i, pr) == (2, 1):
                        # anchored barrier: tiny AllReduce gated on this
                        # pair's first xt write aligns the 4 cores ~10us
                        # before the real collective, so the big AllReduce's
                        # peer-sync wait shrinks to residual skew
                        nc.sync.dma_start(out=drum_in[0][:], in_=xt_sl[2][64:65, 1, 1:17])
                        nc.gpsimd.collective_compute(
                            "AllReduce",
                            mybir.AluOpType.add,
                            replica_groups=GROUPS,
                            ins=[drum_in[0][:]],
                            outs=[drum_out[0][:]],
                        )

            # ---- phase C: conv1 tail, AllReduce, BN1, conv2, out ---------
            AR_DT = BF16
            p1sb = [stg.tile([128, S], AR_DT, name=f"p1sb{m}") for m in range(4)]
            partial1 = [dram.tile([256, S], AR_DT, name=f"partial1{a}") for a in range(2)]
            art = [dram.tile([256, S], AR_DT, name=f"art{a}") for a in range(2)]

            def ar_kick(a):
                for m in (2 * a, 2 * a + 1):
                    nc.sync.dma_start(
                        out=partial1[a][(m % 2) * 128 : (m % 2) * 128 + 128, :],
                        in_=p1sb[m],
                    )
                nc.gpsimd.collective_compute(
                    "AllReduce",
                    mybir.AluOpType.add,
                    replica_groups=GROUPS,
                    ins=[partial1[a][:]],
                    outs=[art[a][:]],
                )

            for m in range(2):
                for n in range(2):
                    conv1_block(2, m, n, final_dst=p1sb[m])
            ar_kick(0)
            for m in range(2, 4):
                for n in range(2):
                    conv1_block(2, m, n, final_dst=p1sb[m])
            ar_kick(1)

            arraw = stg.tile([128, 4, S], AR_DT, name="arraw")
            oout = stg.tile([128, S], F32, name="oout")
            ps2 = [smps.tile([128, 512], F32, name=f"cv2_{n}", tag="sm") for n in range(2)]

            def h1_make(k):
                nc.gpsimd.dma_start(
                    out=arraw[:, k, :],
                    in_=art[k // 2][(k % 2) * 128 : (k % 2) * 128 + 128, :],
                )
                nc.scalar.activation(
                    out=h1_sl[k][:, 1:33, 1:33],
                    in_=arraw[:, k, :].rearrange("p (a b) -> p a b", b=32),
                    func=AF.Relu,
                    bias=avec[:, 4 + k : 5 + k],
                    scale=avec[:, k : k + 1],
                )

            def conv2_k(k):
                for n in range(2):
                    for dy in range(3):
                        for dx in range(3):
                            nc.tensor.matmul(
                                ps2[n],
                                lhsT=c2w_sb[k][:, dy * 3 + dx, :],
                                rhs=h1_sl[k][
                                    :, 16 * n + dy : 16 * n + dy + 16, dx : dx + 32
                                ],
                                start=(k == 0 and dy == 0 and dx == 0),
                                stop=(k == 3 and dy == 2 and dx == 2),
                            )

            for k in range(4):
                h1_make(k)
                conv2_k(k)
            for n in range(2):
                nc.scalar.activation(
                    out=oout[:, 512 * n : 512 * (n + 1)],
                    in_=ps2[n],
                    func=AF.Relu,
                    bias=avec[:, 9:10],
                    scale=avec[:, 8:9],
                )
                nc.sync.dma_start(
                    out=out_d[:, 512 * n : 512 * (n + 1)],
                    in_=oout[:, 512 * n : 512 * (n + 1)],
                )

    nc.finalize()
    return nc


def _f(x):
    return np.ascontiguousarray(x, dtype=np.float32)


def _bf(x):
    return np.ascontiguousarray(np.asarray(x, dtype=np.float32).astype(ml_dtypes.bfloat16))


def prepare_core_inputs(inp):
    """Build the 8 per-core input dicts from the full-problem inputs."""
    inp = {k: np.asarray(v, dtype=np.float64) for k, v in inp.items()}
    x = inp["x"].reshape(B, C, S)
    xp = inp["x_prev"].reshape(B, C, S)
    xn = inp["x_next"].reshape(B, C, S)

    bn1s_full = inp["bn1g"] / np.sqrt(inp["bn1v"] + EPS)
    bn1b_full = inp["bn1b"] - inp["bn1m"] * bn1s_full
    bn2s_full = inp["bn2g"] / np.sqrt(inp["bn2v"] + EPS)
    bn2b_full = inp["bn2b"] - inp["bn2m"] * bn2s_full

    per_g = []
    for g in range(4):
        sl = slice(128 * g, 128 * (g + 1))
        wqT = np.stack(
            [
                np.stack([inp["Wq"][i][sl, 128 * k : 128 * (k + 1)].T for k in range(4)])
                for i in range(3)
            ]
        )
        wkT = np.stack(
            [
                np.stack([inp["Wk"][i][sl, 128 * k : 128 * (k + 1)].T for k in range(4)])
                for i in range(3)
            ]
        )
        bqv = np.stack([inp["bq"][i][sl] for i in range(3)], axis=1)
        bkv = np.stack([inp["bk"][i][sl] for i in range(3)], axis=1)

        att_s = np.stack(
            [inp["bng"][i][sl] / np.sqrt(inp["bnv"][i][sl] + EPS) for i in range(3)]
        )  # (3,128)
        # xtb: post-BN channel bias, folded into the value weights (the
        # attention rows sum to 1 after normalization, so adding xtb to every
        # key position of u produces  y/den + xtb  for free)
        xtb = np.stack(
            [
                inp["bnb"][i][sl] + (inp["bo"][i][sl] - inp["bnm"][i][sl]) * att_s[i]
                for i in range(3)
            ]
        )  # (3,128)

        wvo_rows = []
        wobv_row = []
        for i in range(3):
            for hl in range(4):
                hg = 4 * g + hl
                wv_h = inp["Wv"][i][32 * hg : 32 * (hg + 1), :]  # (32, 512)
                bv_h = inp["bv"][i][32 * hg : 32 * (hg + 1)]
                wo_h = inp["Wo"][i, hg]  # (32, 32)
                sc = att_s[i][32 * hl : 32 * (hl + 1)]  # (32,)
                wvo_rows.append(sc[:, None] * (wo_h @ wv_h))
                wobv_row.append(sc * (wo_h @ bv_h) + xtb[i][32 * hl : 32 * (hl + 1)])
        wvo_all = np.concatenate(wvo_rows, axis=0)  # (384, 512)
        wobv = np.concatenate(wobv_row)[None, :]  # (1, 384)
        wvoT = np.stack([wvo_all[:, 128 * k : 128 * (k + 1)].T for k in range(4)])

        c1wT = np.stack(
            [
                np.stack(
                    [
                        inp["c1w"][
                            128 * m : 128 * (m + 1),
                            512 * i + 128 * g : 512 * i + 128 * (g + 1),
                        ]
                        .transpose(1, 2, 3, 0)
                        .reshape(128, 9, 128)
                        for m in range(4)
                    ]
                )
                for i in range(3)
            ]
        )
        c2wT = np.stack(
            [
                inp["c2w"][sl, 128 * k : 128 * (k + 1)]
                .transpose(1, 2, 3, 0)
                .reshape(128, 9, 128)
                for k in range(4)
            ]
        )
        avec = np.concatenate(
            [
                bn1s_full.reshape(4, 128).T,
                bn1b_full.reshape(4, 128).T,
                bn2s_full[sl][:, None],
                bn2b_full[sl][:, None],
            ],
            axis=1,
        )  # (128, 10)

        per_g.append(
            dict(
                wqT=_bf(wqT), wkT=_bf(wkT), wvoT=_bf(wvoT),
                wobv=_f(wobv), c1wT=_bf(c1wT), c2wT=_bf(c2wT),
                dvec=_f(np.concatenate([bqv, bkv], axis=1)),
                avec=_f(avec),
            )
        )

    in_maps = []
    for c in range(NCORES):
        b, g = c % 2, c // 2
        d = dict(per_g[g])
        d["x4"] = _bf(x[b].reshape(4, 128, S))
        d["oth"] = _bf(np.stack([xn[b].reshape(4, 128, S), xp[b].reshape(4, 128, S)]))
        in_maps.append(d)
    return in_maps


_NC_CACHE = {}


def get_nc():
    if "nc" not in _NC_CACHE:
        _NC_CACHE["nc"] = build_nc()
    return _NC_CACHE["nc"]


def assemble(results):
    out = np.zeros((B, C, H, W), dtype=np.float32)
    for c in range(NCORES):
        b, g = c % 2, c // 2
        out[b, 128 * g : 128 * (g + 1)] = results[c]["out"].reshape(128, H, W)
    return out


def kernel(**inputs):
    nc = get_nc()
    in_maps = prepare_core_inputs(inputs)
    res = run_bass_kernel_spmd(nc, in_maps, list(range(NCORES)))
    return assemble(res.results)
